# revision 41
# baseline (speedup 1.0000x reference)
"""Causal self-attention (query-axis softmax) for Trainium2, 8 NeuronCores.

Sharding: 8 cores = 4 batches x 2 half-head-groups. Core c handles batch
c//2 and heads (c%2)*6 .. (c%2)*6+5. Each core computes its heads' full
attention plus its partial output projection; the host sums the two
partials per batch and adds b_proj.

Layout strategy per core (T=2048, C=768, 6 heads = 3 pairs, hd=64):
  - inputs stream in as fp16 (halves HBM traffic; 11-bit mantissa keeps
    QKV error well under the gate); x arrives pre-transposed so the C
    contraction lands on SBUF partitions. fp16 weights also enable FWL,
    hiding every LDWEIGHTS behind the previous matmul.
  - dependency-free dummy matmuls at t=0, between upfront groups, and at
    pair transitions keep the PE busy across DMA waits so HAM never
    clock-gates it back to 1.2 GHz; the exp table set is preloaded by a
    throwaway activation during the initial DMA wait.
  - phase structure is pipelined per head-pair so ScalarE (the softmax
    bottleneck) starts as soon as pair 0's Q/K exist:
      upfront:  Q pair 0 (chunks 0,1) + K chunk 0 + V tiles 0-2
      pair hp:  attention (32 iterations of S^T -> exp -> AV), with
                V tiles 3-15 (hp=0), this pair's late K chunks, the next
                pair's Q/K (hp<2), and the per-tile output projection
                (hp=2) injected into the loop. Injections REUSE the
                iteration's exp-read s_ps tile (a fresh pool acquisition
                would break psS double-buffering and serialize exp<->S^T)
                and are placed by a greedy deadline scheduler that
                prefers iterations whose tile has a disjoint free region.
  - softmax over q (free axis): exp+rowsum fused on ScalarE via
    accum_out over 1024-wide PSUM chunks; normalization folded into V
    rows (scale V[k,:] by 1/denom[k]) on VectorE; at/vsp in bf16 (AV
    matmul bf16 runs 1 cycle/col at any chunk width).
  - causal mask: ragged chunk bounds skip fully-masked blocks; the
    diagonal 128x128 block's triangular -30000 add is folded into the
    S^T accumulation group as an `identity @ mask` matmul on the tensor
    engine, keeping the S^T -> exp chain off VectorE entirely.
  - y^T for q-tile tt is final right after kt=tt's AV lands, so y copies
    spread across every pair's loop and pair 2 projects and DMAs tiles
    0-12 in place; only tiles 13-15 tail.
"""

import os
import sys

sys.path.insert(0, "/opt/trn_rl_repo")

import numpy as np

import concourse.bass as bass
import concourse.mybir as mybir
import concourse.tile as tile
from concourse.bass_utils import run_bass_kernel_spmd

FP32 = mybir.dt.float32
FP32R = mybir.dt.float32r
BF16 = mybir.dt.bfloat16
FP16 = mybir.dt.float16
U16 = mybir.dt.uint16

B, T, C, H = 4, 2048, 768, 12
D = 64                  # head dim
NCORES = 8
HPC = H * B // NCORES   # heads per core = 6
E = HPC * D             # qkv slice width per core = 384
CT = C // 128           # c tiles = 6
ET = E // 128           # e tiles (head pairs) = 3
TT = T // 128           # t tiles = 16
QCH = 512               # matmul moving chunk (PSUM bank limit)
NQC = T // QCH          # 4
BCH = 1024              # exp chunk
NBC = T // BCH          # 2
MASKV = -30000.0
SCALE = 1.0 / 8.0       # 1/sqrt(hd)
Exp = mybir.ActivationFunctionType.Exp


def _av_chunks(klo):
    """Split [klo, T) into matmul chunks of <=512 output cols (PSUM bank
    limit); bf16 operands run 1 cycle/col at any width so the remainder
    chunk can lead."""
    out, pos = [], klo
    r = (T - klo) % 512
    if r:
        out.append((pos, pos + r))
        pos += r
    while pos < T:
        out.append((pos, pos + 512))
        pos += 512
    return out


def _split_sync_waits(nc):
    """This container's walrus encodes at most one sync wait per
    instruction for several instruction structs; hoist extra waits onto
    same-engine nops placed immediately before the instruction."""
    for f in nc.m.functions:
        for bb in f.blocks:
            new_insts = []
            for inst in bb.instructions:
                si = inst.sync_info
                waits = list(si.on_wait) if si is not None and si.on_wait else []
                if len(waits) > 1:
                    for w in waits[:-1]:
                        nop = mybir.InstNoOp(
                            name=nc.get_next_instruction_name(),
                            engine=inst.engine,
                            sync_info=mybir.SyncInfo(on_wait=[w], on_update=[]),
                            bass_nofuse=True,
                        )
                        nc.register_instruction(nop)
                        new_insts.append(nop)
                    inst.sync_info = mybir.SyncInfo(
                        on_wait=[waits[-1]], on_update=list(si.on_update or [])
                    )
                new_insts.append(inst)
            bb.instructions[:] = new_insts


def _build():
    nc = bass.Bass("TRN2")
    xT = nc.dram_tensor("xT", [NQC, 128, CT, QCH], FP16, kind="ExternalInput")
    wq = nc.dram_tensor("wq", [128, CT, E], FP16, kind="ExternalInput")
    wk = nc.dram_tensor("wk", [128, CT, E], FP16, kind="ExternalInput")
    wv = nc.dram_tensor("wv", [128, CT, E], FP16, kind="ExternalInput")
    bq = nc.dram_tensor("bq", [E], FP32, kind="ExternalInput")
    bk = nc.dram_tensor("bk", [E], FP32, kind="ExternalInput")
    bv = nc.dram_tensor("bv", [E], FP32, kind="ExternalInput")
    wp = nc.dram_tensor("wp", [128, ET, C], FP16, kind="ExternalInput")
    mask = nc.dram_tensor("mask", [128, 128], FP16, kind="ExternalInput")
    ident = nc.dram_tensor("ident", [128, 128], FP16, kind="ExternalInput")
    out = nc.dram_tensor("out", [T, C], FP16, kind="ExternalOutput")

    with tile.TileContext(nc) as tc:
        with (
            tc.tile_pool(name="wts", bufs=1) as wts,
            tc.tile_pool(name="big", bufs=1) as big,
            tc.tile_pool(name="atp", bufs=6) as atp,
            tc.tile_pool(name="sm", bufs=4) as sm,
            tc.tile_pool(name="op", bufs=3) as op,
        ):
            # ---- PE warm-up (see module docstring) ----
            warm_sb = wts.tile([128, QCH], BF16, name="warm_sb")
            nc.vector.memset(warm_sb.bitcast(U16), 0)
            # preload the exp table set (~2.7us) while DMAs stream
            warm_act = wts.tile([128, 1], FP32, name="warm_act")
            nc.scalar.activation(warm_act, warm_sb[:, 0:1], Exp)
            with tc.tile_pool(name="psW", bufs=1, space="PSUM") as psW:
                warm_ps = psW.tile([128, QCH], FP32, name="warm_ps")
                for _ in range(30):
                    nc.tensor.matmul(
                        warm_ps, warm_sb[:, 0:128], warm_sb, start=True, stop=True
                    )

            qt2 = big.tile([128, ET, T], FP16)      # [d-in-pair, pair, t]
            ktp2 = big.tile([128, ET, 2, T], FP16)  # [d(+zero half), pair, head-in-pair, t]
            v_sb = big.tile([128, TT, E], FP32)     # [t-in-tile, ttile, (head,d)]
            y_sb = big.tile([128, ET, T], FP16)     # [hd-in-pair, pair, t]
            # persistent rotating Vs tiles, 3 per head-in-pair slot; head A
            # tiles keep cols 64:128 zero, head B tiles keep cols 0:64 zero.
            vspad = [
                [big.tile([128, 128], BF16, name=f"vspad{j}_{i}") for i in range(4)]
                for j in range(2)
            ]

            # ---- input loads, first-use order, 3 HWDGE queues ----
            dmae = [nc.sync, nc.scalar, nc.gpsimd]
            nq = len(dmae)
            qi = 0

            def ld(dst, src):
                nonlocal qi
                dmae[qi % nq].dma_start(out=dst, in_=src)
                qi += 1

            # Order matches first compute use: the first S^T needs Q0 c0/c1
            # (wq + xts0/xts1) and K0 c0 (wk), then Q0 c2/c3 (xts2/xts3),
            # then V (wv). Small biases/mask ride the direct queues early.
            xts = [
                big.tile([128, CT, QCH], FP16, name=f"xts{i}") for i in range(NQC)
            ]
            wq_sb = wts.tile([128, CT, E], FP16)
            wk_sb = wts.tile([128, CT, E], FP16)
            wv_sb = wts.tile([128, CT, E], FP16)
            bq_sb = wts.tile([128, ET], FP32)
            bk_sb = wts.tile([128, ET], FP32)
            nc.sync.dma_start(out=bq_sb, in_=bq.rearrange("(et p) -> p et", p=128))
            nc.scalar.dma_start(out=bk_sb, in_=bk.rearrange("(et p) -> p et", p=128))
            mask_sb = wts.tile([128, 128], FP16)
            nc.scalar.dma_start(out=mask_sb, in_=mask[:])
            ident_sb = wts.tile([128, 128], FP16)
            nc.sync.dma_start(out=ident_sb, in_=ident[:])
            for ct in range(CT):
                ld(wq_sb[:, ct, :], wq[:, ct, :])
                ld(xts[0][:, ct, :], xT[0, :, ct, :])
            for ct in range(CT):
                ld(wk_sb[:, ct, :], wk[:, ct, :])
                ld(xts[1][:, ct, :], xT[1, :, ct, :])
            bv_sb = wts.tile([128, E], FP32)
            nc.sync.dma_start(out=bv_sb, in_=bv[None, :].to_broadcast((128, E)))
            # xts2/xts3 stay in the critical prefix: iteration 0's Q0c2/c3
            # and the exp over q[1024:2048) need them, and delaying them
            # stalled the in-order tensor queue long enough to re-throttle
            # the PE.
            for ct in range(CT):
                ld(xts[2][:, ct, :], xT[2, :, ct, :])
            for ct in range(CT):
                ld(xts[3][:, ct, :], xT[3, :, ct, :])
            # ktp2 zero-halves (the K bias-adds never write these) + vspad
            # zeroing sit between gpsimd's critical-load issues and its
            # late-load issues: the ~12us of memsets both do required work
            # and delay the late descriptors so the DMA engines drain the
            # critical prefix first.
            nc.gpsimd.memset(ktp2[64:128, :, 0, :].bitcast(U16), 0)
            nc.gpsimd.memset(ktp2[0:64, :, 1, :].bitcast(U16), 0)
            for row in vspad:
                for t_ in row:
                    nc.gpsimd.memset(t_.bitcast(U16), 0)
            # Only wv and wp ride the delayed late group (first needed at
            # ~24us and ~170us respectively).
            for ct in range(CT):
                nc.gpsimd.dma_start(out=wv_sb[:, ct, :], in_=wv[:, ct, :])
            wp_sb = wts.tile([128, ET, C], FP16)
            nc.gpsimd.dma_start(out=wp_sb, in_=wp[:])


            # ---- helpers shared by the upfront block and injections ----
            def emit_q(ps, e1, tci):
                cols = slice(tci * QCH, (tci + 1) * QCH)
                for ct in range(CT):
                    nc.tensor.matmul(
                        ps[:, 0:QCH], wq_sb[:, ct, e1 * 128:(e1 + 1) * 128],
                        xts[tci][:, ct, :],
                        start=(ct == 0), stop=(ct == CT - 1),
                    )
                nc.vector.tensor_scalar_add(
                    qt2[:, e1, cols], ps[:, 0:QCH], bq_sb[:, e1:e1 + 1]
                )

            def emit_k(ps, e1, tci):
                cols = slice(tci * QCH, (tci + 1) * QCH)
                for ct in range(CT):
                    nc.tensor.matmul(
                        ps[:, 0:QCH], wk_sb[:, ct, e1 * 128:(e1 + 1) * 128],
                        xts[tci][:, ct, :],
                        start=(ct == 0), stop=(ct == CT - 1),
                    )
                nc.vector.tensor_scalar_add(
                    ktp2[0:64, e1, 0, cols], ps[0:64, 0:QCH], bk_sb[0:64, e1:e1 + 1]
                )
                nc.vector.tensor_scalar_add(
                    ktp2[64:128, e1, 1, cols], ps[64:128, 0:QCH], bk_sb[64:128, e1:e1 + 1]
                )

            def emit_v(ps, tt):
                tci, ttl = tt // 4, tt % 4
                for ct in range(CT):
                    nc.tensor.matmul(
                        ps[:, :E], xts[tci][:, ct, ttl * 128:(ttl + 1) * 128],
                        wv_sb[:, ct, :],
                        start=(ct == 0), stop=(ct == CT - 1),
                    )
                nc.vector.tensor_add(v_sb[:, tt, :], ps[:, :E], bv_sb)

            # ---- upfront: Q for pair 0 (all T) + K chunk 0 + V tiles 0..2,
            # in DMA-arrival order so compute tracks the input stream ----
            with tc.tile_pool(name="psA", bufs=4, space="PSUM") as psA:
                def _ps(nm):
                    return psA.tile([128, QCH], FP32, tag="ps", bufs=4, name=nm)

                def _bridge(n):
                    # dependency-free dummies between upfront groups keep
                    # the PE busy across their DMA waits so HAM stays warm
                    ps = _ps("pw")
                    for _ in range(n):
                        nc.tensor.matmul(
                            ps, warm_sb[:, 0:128], warm_sb, start=True, stop=True
                        )
                _bridge(5)
                emit_q(_ps("pq"), 0, 0)
                _bridge(6)
                emit_q(_ps("pq"), 0, 1)
                _bridge(6)
                emit_k(_ps("pk"), 0, 0)
                _bridge(5)

            # ---- injection slot scheduler ----
            # Injected work (V tiles, next pair's Q/K, output projection)
            # REUSES the iteration's already-read s_ps tile instead of
            # acquiring a new pool slot: an extra acquisition per iteration
            # would break the psS double-buffer and serialize exp <-> S^T.
            # The injected matmuls write cols [0, w) of the tile; WAR deps
            # on the exp read order them, and on iterations where [0, w)
            # is disjoint from the exp-read region they run concurrently
            # with the exp. A greedy deadline pass assigns each job the
            # latest feasible slot, preferring disjoint ones.
            def _free_width(j):
                """Cols of the reused tile not read by its exp."""
                kt = j // 2
                return 128 * kt if kt < 8 else 128 * kt - BCH

            def _schedule(jobs):
                """jobs: (deadline_j, width, release_j, fn). Returns
                slot->fn map. Latest feasible slot per job; disjoint
                preferred; tighter (deadline, -release) jobs pick first."""
                taken = {}
                for dl, w, rel, fn in sorted(jobs, key=lambda x: (x[0], -x[2])):
                    best = None
                    for j in range(min(dl, 31), rel - 1, -1):
                        if j in taken:
                            continue
                        if _free_width(j) >= w:
                            best = j
                            break
                        if best is None:
                            best = j
                    assert best is not None, f"no slot for job dl={dl}"
                    taken[best] = fn
                return taken

            # ---- attention, pair-pipelined with injections ----
            # The two heads of a pair interleave their kt loops and SHARE
            # one y^T psum tile: head A's padded AV writes zeros to rows
            # 64:128 (B's rows) and vice versa, and accumulating zero is a
            # no-op, so rows 0:64 = head A's y^T and 64:128 = head B's.
            with (
                tc.tile_pool(name="psS", bufs=2, space="PSUM") as psS,
                tc.tile_pool(name="psY", bufs=1, space="PSUM") as psY,
            ):
                yps_ref = [None]

                def emit_proj(tt, ps=None):
                    """Project q-tile tt (needs all three pairs' y) and DMA
                    it out. Copies stay off ScalarE — it is the softmax
                    bottleneck during pair 2 where these are injected."""
                    nc.vector.tensor_copy(
                        y_sb[:, 2, tt * 128:(tt + 1) * 128],
                        yps_ref[0][:, tt * 128:(tt + 1) * 128],
                    )
                    if ps is None:
                        ps = psS.tile([128, BCH], FP32, tag="s", bufs=2, name="s_ps")
                    for et in range(ET):
                        nc.tensor.matmul(
                            ps[:, 0:QCH], y_sb[:, et, tt * 128:(tt + 1) * 128],
                            wp_sb[:, et, 0:QCH],
                            start=(et == 0), stop=(et == ET - 1),
                        )
                        nc.tensor.matmul(
                            ps[:, QCH:C], y_sb[:, et, tt * 128:(tt + 1) * 128],
                            wp_sb[:, et, QCH:C],
                            start=(et == 0), stop=(et == ET - 1),
                        )
                    o_sb = op.tile([128, C], FP16, tag="o", bufs=3, name="o_sb")
                    nc.vector.tensor_copy(o_sb, ps[:, 0:C])
                    nc.sync.dma_start(out=out[tt * 128:(tt + 1) * 128, :], in_=o_sb)

                for hp in range(ET):
                    # build this pair's injection schedule
                    jobs = []
                    # late K chunks for THIS pair: K chunk c feeds kt=4c,
                    # first used at j=8c; land 2 iterations early.
                    if hp > 0:
                        for c in (1, 2, 3):
                            jobs.append((8 * c - 2, QCH, 0,
                                         (lambda c=c: lambda ps: emit_k(ps, hp, c))()))
                    if hp == 0:
                        for c in (1, 2, 3):
                            jobs.append((8 * c - 2, QCH, 0,
                                         (lambda c=c: lambda ps: emit_k(ps, 0, c))()))
                        # V tiles 3..15, consumed by the vsp scale at j=2tt
                        for tt in range(3, TT):
                            jobs.append((2 * tt - 2, E, 0,
                                         (lambda tt=tt: lambda ps: emit_v(ps, tt))()))
                    if hp < 2:
                        # next pair's Q (all chunks) + K chunk 0: needed at
                        # the next pair's j=0. Staggered deadlines spread
                        # them across the pair so the PE never idles long
                        # enough (>3.4us) for HAM to re-throttle.
                        for c, dl in zip(range(NQC), (12, 18, 24, 28)):
                            jobs.append((dl, QCH, 8,
                                         (lambda c=c: lambda ps: emit_q(ps, hp + 1, c))()))
                        jobs.append((30, QCH, 8,
                                     (lambda: lambda ps: emit_k(ps, hp + 1, 0))()))
                    if hp == 2:
                        # per-tile projection: y^T[:, tt] is final once
                        # kt=tt's AV has landed (AV(tt, hj=1) is emitted at
                        # iteration 2tt+4). Tiles 13..15 finalize too late
                        # for in-loop slots; the tail below handles them.
                        for tt in range(14):
                            jobs.append((min(2 * tt + 9, 31), C, min(2 * tt + 5, 31),
                                         (lambda tt=tt: lambda ps: emit_proj(tt, ps))()))
                    slots = _schedule(jobs)

                    if hp > 0:
                        dm_ps = psS.tile([128, BCH], FP32, tag="s", bufs=2, name="s_ps")
                        for _ in range(5):
                            nc.tensor.matmul(
                                dm_ps[:, 0:QCH], warm_sb[:, 0:128], warm_sb,
                                start=True, stop=True,
                            )
                    yps = psY.tile([128, T], FP32, tag="y", name="yps")
                    yps_ref[0] = yps
                    pend = []  # [(hj, kt, at, vsp)] awaiting AV emission

                    def emit_av(hj, kt0, at0, vsp0):
                        for lo, hi in _av_chunks(128 * kt0):
                            nc.tensor.matmul(
                                yps[:, lo:hi], vsp0, at0[:, lo:hi],
                                start=(kt0 == 0 and hj == 0),
                                stop=(kt0 == TT - 1 and hj == 1),
                                skip_group_check=True,
                            )

                    for kt in range(TT):
                        klo = 128 * kt
                        bc0 = klo // BCH
                        for hj in range(2):
                            j = 2 * kt + hj
                            hl = 2 * hp + hj
                            at = atp.tile([128, T], BF16, tag="at", bufs=6, name="at")
                            sums = sm.tile([128, NBC], FP32, tag="sums", bufs=4, name="sums")
                            inj_tile = None
                            for bc in range(bc0, NBC):
                                blo = max(BCH * bc, klo)
                                s_ps = psS.tile([128, BCH], FP32, tag="s", bufs=2, name="s_ps")
                                if inj_tile is None:
                                    inj_tile = s_ps
                                for half in range(2):
                                    plo = max(blo, BCH * bc + half * QCH)
                                    phi = BCH * bc + (half + 1) * QCH
                                    if plo >= phi:
                                        continue
                                    diag = bc == bc0 and plo == klo
                                    nc.tensor.matmul(
                                        s_ps[:, plo - BCH * bc:phi - BCH * bc],
                                        ktp2[:, hp, hj, klo:klo + 128],
                                        qt2[:, hp, plo:phi],
                                        start=True, stop=not diag,
                                    )
                                    if diag:
                                        off = klo - BCH * bc
                                        nc.tensor.matmul(
                                            s_ps[:, off:off + 128],
                                            ident_sb, mask_sb,
                                            start=False, stop=True,
                                        )
                                if hp == 0 and j == 0 and bc == 0:
                                    for c in (2, 3):
                                        emit_q(
                                            psS.tile([128, BCH], FP32, tag="s", bufs=2, name="s_ps"),
                                            0, c,
                                        )
                                nc.scalar.activation(
                                    at[:, blo:BCH * (bc + 1)],
                                    s_ps[:, blo - BCH * bc:],
                                    Exp, scale=SCALE, accum_out=sums[:, bc:bc + 1],
                                )
                            if hp == 0 and j == 0:
                                for tt in range(3):
                                    emit_v(
                                        psS.tile([128, BCH], FP32, tag="s", bufs=2, name="s_ps"),
                                        tt,
                                    )
                            rcp = sm.tile([128, 1], FP32, tag="rcp", bufs=4, name="rcp")
                            if bc0 == NBC - 1:
                                nc.vector.reciprocal(rcp, sums[:, bc0:NBC])
                            else:
                                stot = sm.tile([128, 1], FP32, tag="stot", bufs=4, name="stot")
                                nc.vector.reduce_sum(stot, sums[:, bc0:NBC], axis=mybir.AxisListType.X)
                                nc.vector.reciprocal(rcp, stot)
                            vsp = vspad[hj][kt % 4]
                            nc.vector.tensor_scalar_mul(
                                vsp[:, hj * 64:hj * 64 + 64],
                                v_sb[:, kt, hl * 64:(hl + 1) * 64], rcp
                            )
                            pend.append((hj, kt, at, vsp))
                            if len(pend) > 3:
                                emit_av(*pend.pop(0))
                            # guaranteed-runnable AV is emitted above; the
                            # injection reuses this iteration's first s_ps
                            # tile, so it can't steal pipeline depth.
                            if j in slots:
                                slots[j](inj_tile)
                            if hp < 2 and j % 2 == 0 and j >= 8:
                                tt = (j - 8) // 2
                                nc.vector.tensor_copy(
                                    y_sb[:, hp, tt * 128:(tt + 1) * 128],
                                    yps[:, tt * 128:(tt + 1) * 128],
                                )
                    for p_ in pend:
                        emit_av(*p_)
                    if hp < 2:
                        nc.vector.tensor_copy(y_sb[:, hp, 12 * 128:], yps[:, 12 * 128:])

                # ---- tail: projection of the last q-tiles (their AVs only
                # land in the pend flush above) ----
                for tt in range(14, TT):
                    emit_proj(tt)

    _split_sync_waits(nc)
    return nc


_nc_cache = {}
last_result = None


def kernel(x, w_attn, b_attn, w_proj, b_proj):
    global last_result
    if "nc" not in _nc_cache:
        _nc_cache["nc"] = _build()
    nc = _nc_cache["nc"]

    x = np.asarray(x, dtype=np.float16)
    w_attn = np.asarray(w_attn, dtype=np.float16)
    b_attn = np.asarray(b_attn, dtype=np.float32)
    w_proj = np.asarray(w_proj, dtype=np.float16)
    b_proj = np.asarray(b_proj, dtype=np.float32)

    tri = np.where(
        np.arange(128)[None, :] >= np.arange(128)[:, None], 0.0, MASKV
    ).astype(np.float16)
    ident = np.eye(128, dtype=np.float16)

    in_maps = []
    for core in range(NCORES):
        b = core // 2
        e0 = (core % 2) * E
        xt_host = np.ascontiguousarray(
            x[b].T.reshape(CT, 128, NQC, QCH).transpose(2, 1, 0, 3)
        )
        def _wblk(w):
            return np.ascontiguousarray(w.reshape(CT, 128, E).transpose(1, 0, 2))
        in_maps.append({
            "xT": xt_host,
            "wq": _wblk(w_attn[:, e0:e0 + E]),
            "wk": _wblk(w_attn[:, C + e0:C + e0 + E]),
            "wv": _wblk(w_attn[:, 2 * C + e0:2 * C + e0 + E]),
            "bq": np.ascontiguousarray(b_attn[e0:e0 + E]),
            "bk": np.ascontiguousarray(b_attn[C + e0:C + e0 + E]),
            "bv": np.ascontiguousarray(b_attn[2 * C + e0:2 * C + e0 + E]),
            "wp": np.ascontiguousarray(
                w_proj[e0:e0 + E, :].reshape(ET, 128, C).transpose(1, 0, 2)
            ),
            "mask": tri,
            "ident": ident,
        })

    trace = os.environ.get("ATT_TRACE", "0")
    kw = {}
    if trace != "0":
        n = min(int(trace), NCORES)
        kw = dict(trace=True, trace_cores=list(range(n)))
    res = run_bass_kernel_spmd(nc, in_maps, list(range(NCORES)), **kw)
    last_result = res

    out = np.zeros((B, T, C), dtype=np.float32)
    for core in range(NCORES):
        out[core // 2] += res.results[core]["out"].astype(np.float32)
    out += b_proj[None, None, :]
    return out
